# revision 1
# baseline (speedup 1.0000x reference)
"""Trainium2 Bass kernel for nn_ConvAlignLoss (8-core data parallel).

Self-contained: hardcodes shapes; imports concourse from /opt/trn_rl_repo.

Per core (R=64 rows):
  loss_astf partial: sum((pred-true)^2)
  conv = irfft16384(fft(pred) * conj(fft(egf_pad)))[:14337]  (2-stage matmul FFT)
  cc   = irfft32768(fft(conv_pad) * conj(fft(target_pad)))
  shift = mapped masked argmax of cc (== reference argmax over n=28673)
  loss_conv partial: sum((conv[(7040+i+shift) % 14337] - target[7040+i])^2)
Host combines the 8 cores' (sum_astf, sum_conv) into the scalar losses.

FFT structure (N = 128*N2), Hermitian-cropped for real inputs:
  FWD:  D[t1,t2]=x[N2*t1+t2]; A[t2,f1]=sum_t1 D*W1[:, :65]; B=A*tw;
        Z[f2,f1]=sum_t2 W2[t2,f2]*B  -- only f1 in [0,64] kept.
  INV (Hermitian S):  G[f1,t2]=sum_f2 S2d[f2,f1]*V2[f2,t2]; H=G*itw;
        x2d[t1,t2]=Re(sum_{f1<=64} c_f1 V1[f1,t1]*H[f1,t2])/N,
        c_f1 = 1 for f1 in {0,64}, else 2.
"""
import sys

sys.path.insert(0, "/opt/trn_rl_repo")

import numpy as np
import concourse.bass as bass
import concourse.bacc as bacc
import concourse.mybir as mybir
from concourse import tile

F32 = mybir.dt.float32
BF16 = mybir.dt.bfloat16
I32 = mybir.dt.int32
AT = mybir.AluOpType
AX = mybir.AxisListType

R = 64
NCORES = 8
L1, L2 = 16384, 2048
CONV_LEN = L1 - L2 + 1      # 14337
N_A, N_B = 16384, 32768
GAP_LO, GAP_HI = CONV_LEN, N_B - CONV_LEN + 1   # gap [14337, 18432)
CROP = 256
START0 = (CONV_LEN - CROP) // 2                 # 7040
PITCH = 14720
BIGL = float(2 ** 23)
F1 = 65                      # Hermitian half: f1 in [0, 64]


def _dft(n, sign):
    k = np.arange(n)
    return np.exp(sign * 2j * np.pi * np.outer(k, k) / n)


def make_consts():
    c = {}

    def put(name, arr, dt=np.float32):
        c[name] = np.ascontiguousarray(np.asarray(arr, np.float64)).astype(dt)

    cf = np.ones(F1)
    cf[1:64] = 2.0

    W1 = _dft(128, -1)
    put("W1r", W1.real); put("W1i", W1.imag); put("nW1i", -W1.imag)
    put("W1r65", W1.real[:, :F1]); put("W1i65", W1.imag[:, :F1])
    twA = np.exp(-2j * np.pi * np.outer(np.arange(128), np.arange(F1)) / N_A)
    put("twAr", twA.real); put("twAi", twA.imag)
    V2A = _dft(128, +1)
    put("V2Ar", V2A.real); put("V2Ai", V2A.imag); put("nV2Ai", -V2A.imag)
    itwA = np.exp(2j * np.pi * np.outer(np.arange(F1), np.arange(128)) / N_A)
    put("itwAr", itwA.real); put("itwAi", itwA.imag)
    V1A = cf[:, None] * _dft(128, +1)[:F1] / N_A        # [f1<=64, t1]
    put("V1Ar", V1A.real); put("nV1Ai", -V1A.imag)

    W2B = _dft(256, -1)          # [t2, f2]
    for a in range(2):
        for b in range(2):
            blk = W2B[a * 128:(a + 1) * 128, b * 128:(b + 1) * 128]
            put(f"W2Br{a}{b}", blk.real)
            put(f"W2Bi{a}{b}", blk.imag)
            put(f"nW2Bi{a}{b}", -blk.imag)
    twB = np.exp(-2j * np.pi * np.outer(np.arange(256), np.arange(F1)) / N_B)
    for a in range(2):
        put(f"twBr{a}", twB.real[a * 128:(a + 1) * 128])
        put(f"twBi{a}", twB.imag[a * 128:(a + 1) * 128])
    V2B = _dft(256, +1)          # [f2, t2]
    for a in range(2):
        blk = V2B[a * 128:(a + 1) * 128, :]
        put(f"V2Br{a}", blk.real)
        put(f"V2Bi{a}", blk.imag)
        put(f"nV2Bi{a}", -blk.imag)
    itwB = np.exp(2j * np.pi * np.outer(np.arange(F1), np.arange(256)) / N_B)
    put("itwBr", itwB.real); put("itwBi", itwB.imag)
    V1B = cf[:, None] * _dft(128, +1)[:F1] / N_B        # [f1<=64, t1]
    put("V1Br", V1B.real); put("nV1Bi", -V1B.imag)

    put("ident", np.eye(128))
    put("ones1x128", np.ones((1, 128)))
    put("ones128", np.ones((128, 1)))
    put("ones64", np.ones((64, 1)))

    j = np.arange(128)[:, None] * 256 + np.arange(256)[None, :]   # [t1, t2]
    gap = (j >= GAP_LO) & (j < GAP_HI)
    put("maskB", np.where(gap, -1e30, 0.0))
    shiftval = np.where(j <= CONV_LEN - 1, j - (CONV_LEN - 1), j - GAP_HI + 1)
    put("shvB", np.where(gap, 0.0, shiftval - BIGL))
    put("winidx", np.arange(R)[:, None] * PITCH
        + np.arange(CROP)[None, :])                               # [64, 256]
    return c


def _b3(ap, n, inner):
    """[p, inner] const AP -> [p, n, inner] broadcast over middle dim."""
    return ap.rearrange("p (a b) -> p a b", a=1).to_broadcast(
        [ap.shape[0], n, inner])


def _cmul_psum(nc, pool, eng, outr, outi, pr, pi, twr, twi, inner, n,
               part=128, tag="twtmp", tdt=F32):
    """(outr + i outi) = (pr + i pi) * (twr + i twi); p* in PSUM/SBUF, tw
    const APs broadcast over n blocks of `inner`. outr/outi are SBUF APs
    [part, n*inner]. eng: engine proxy (nc.vector or nc.gpsimd)."""
    tmp = pool.tile([part, n * inner], tdt, tag=tag, name=tag)
    orv = outr.rearrange("p (a b) -> p a b", b=inner)
    oiv = outi.rearrange("p (a b) -> p a b", b=inner)
    prv = pr.rearrange("p (a b) -> p a b", b=inner)
    piv = pi.rearrange("p (a b) -> p a b", b=inner)
    tv = tmp[:].rearrange("p (a b) -> p a b", b=inner)
    eng.tensor_tensor(orv, prv, twr, op=AT.mult)
    eng.tensor_tensor(tv, piv, twi, op=AT.mult)
    eng.tensor_tensor(orv, orv, tv, op=AT.subtract)
    eng.tensor_tensor(oiv, prv, twi, op=AT.mult)
    eng.tensor_tensor(tv, piv, twr, op=AT.mult)
    eng.tensor_tensor(oiv, oiv, tv, op=AT.add)


def _spectral(nc, eng, sr, si, ar, ai, br, bi, tmp):
    """S = A * conj(B): sr = ar*br + ai*bi; si = ai*br - ar*bi."""
    eng.tensor_tensor(sr, ar, br, op=AT.mult)
    eng.tensor_tensor(tmp, ai, bi, op=AT.mult)
    eng.tensor_tensor(sr, sr, tmp, op=AT.add)
    eng.tensor_tensor(si, ai, br, op=AT.mult)
    eng.tensor_tensor(tmp, ar, bi, op=AT.mult)
    eng.tensor_tensor(si, si, tmp, op=AT.subtract)


def _mm_const_names():
    s = {"W1r", "W1i", "nW1i", "W1r65", "W1i65", "V2Ar", "V2Ai", "nV2Ai",
         "V1Ar", "nV1Ai", "V1Br", "nV1Bi"}
    s |= {f"W2Br{a}{b}" for a in range(2) for b in range(2)}
    s |= {f"W2Bi{a}{b}" for a in range(2) for b in range(2)}
    s |= {f"nW2Bi{a}{b}" for a in range(2) for b in range(2)}
    s |= {f"V2Br{a}" for a in range(2)} | {f"V2Bi{a}" for a in range(2)}
    s |= {f"nV2Bi{a}" for a in range(2)}
    # twiddles consumed by DVE cmuls: bf16 for 2x/4x DVE modes
    s |= {"twAr", "twAi", "itwAr", "itwAi", "itwBr", "itwBi"}
    s |= {f"twBr{a}" for a in range(2)} | {f"twBi{a}" for a in range(2)}
    return s


def build_nc(cdt=BF16, rows=R, rbb=8, rb2=4):
    nc = bacc.Bacc("TRN2", target_bir_lowering=False, debug=False,
                   num_devices=NCORES)
    consts = make_consts()

    pred = nc.dram_tensor("pred", [rows, L1], F32, kind="ExternalInput")
    true_ = nc.dram_tensor("true", [rows, L1], F32, kind="ExternalInput")
    egf = nc.dram_tensor("egf", [rows, L2], F32, kind="ExternalInput")
    target = nc.dram_tensor("target", [rows, CONV_LEN], F32,
                            kind="ExternalInput")
    out = nc.dram_tensor("out", [1, 2], F32, kind="ExternalOutput")
    scratch = nc.dram_tensor("scratch", [rows, PITCH], cdt)

    MM_CONST = _mm_const_names()

    cdram = {}
    for name, arr in consts.items():
        cdt_n = cdt if name in MM_CONST else F32
        cdram[name] = nc.dram_tensor(name, list(arr.shape), cdt_n,
                                     kind="ExternalInput")

    nb1, nb2 = rows // rbb, rows // rb2

    with tile.TileContext(nc) as tc:
        with (
            tc.tile_pool(name="consts", bufs=1) as cpool,
            tc.tile_pool(name="keep", bufs=1) as kpool,
            tc.tile_pool(name="ps", bufs=2, space="PSUM") as pp,
        ):
            cs = {}
            for name, arr in consts.items():
                dt = cdt if name in MM_CONST else F32
                t = cpool.tile(list(arr.shape), dt, tag=f"c_{name}", name=f"c_{name}")
                nc.sync.dma_start(t[:], cdram[name][:])
                cs[name] = t

            allmax = kpool.tile([128, rows], F32, tag="allmax", name="allmax")
            allmin = kpool.tile([128, rows], F32, tag="allmin", name="allmin")
            ccm_all = kpool.tile([128, rows * 256], BF16, tag="ccm", name="ccm")
            astf_acc = kpool.tile([128, 8], F32, tag="astfacc", name="astfacc")
            shifts = kpool.tile([rows, 1], F32, tag="shifts", name="shifts")
            outt = kpool.tile([1, 2], F32, tag="outt", name="outt")

            # ---------------- A) astf ----------------
            predf = pred.ap().rearrange("r l -> (r l)").rearrange(
                "(p f) -> p f", p=128)
            truef = true_.ap().rearrange("r l -> (r l)").rearrange(
                "(p f) -> p f", p=128)
            fch = rows * L1 // 128 // 8
            with tc.tile_pool(name="astf", bufs=2) as apool:
                for i in range(8):
                    tp = apool.tile([128, fch], F32, tag="ap", name="ap")
                    tt = apool.tile([128, fch], F32, tag="at", name="at")
                    sl = bass.ts(i, fch)
                    nc.sync.dma_start(tp[:], predf[:, sl])
                    nc.sync.dma_start(tt[:], truef[:, sl])
                    nc.gpsimd.tensor_tensor(tt[:], tp[:], tt[:], op=AT.subtract)
                    nc.vector.scalar_tensor_tensor(
                        tp[:], tt[:], 1.0, tt[:], op0=AT.bypass, op1=AT.mult,
                        accum_out=astf_acc[:, i:i + 1])

            # ---------------- B) 16K level ----------------
            with tc.tile_pool(name="p16", bufs=2) as dp:
                for b in range(nb1):
                    r0 = b * rbb
                    Dp = dp.tile([128, rbb * 128], cdt, tag="Dp", name="Dp")
                    De = dp.tile([16, rbb * 128], cdt, tag="De", name="De")
                    psrc = pred[r0:r0 + rbb, :].rearrange(
                        "q (a b) -> q a b", a=128).transpose([1, 0, 2])
                    esrc = egf[r0:r0 + rbb, :].rearrange(
                        "q (a b) -> q a b", a=16).transpose([1, 0, 2])
                    if cdt == F32:
                        nc.sync.dma_start(
                            Dp[:].rearrange("p (q b) -> p q b", b=128), psrc)
                        nc.sync.dma_start(
                            De[:].rearrange("p (q b) -> p q b", b=128), esrc)
                    else:
                        Dst = dp.tile([128, rbb * 128], F32, tag="Dst", name="Dst")
                        Est = dp.tile([16, rbb * 128], F32, tag="Est", name="Est")
                        nc.sync.dma_start(
                            Dst[:].rearrange("p (q b) -> p q b", b=128), psrc)
                        nc.sync.dma_start(
                            Est[:].rearrange("p (q b) -> p q b", b=128), esrc)
                        nc.gpsimd.tensor_copy(Dp[:], Dst[:])
                        nc.gpsimd.tensor_copy(De[:], Est[:])

                    Bs = {k: dp.tile([128, rbb * F1], cdt, tag=f"B{k}", name=f"B{k}")
                          for k in ("pr", "pi", "er", "ei")}
                    for g in range(rbb // 4):
                        gsl = bass.ts(g, 4 * F1)
                        for inp, D, kp in (("p", Dp, 128), ("e", De, 16)):
                            pa = pp.tile([128, 4 * F1], F32, tag="st1", name="st1")
                            pai = pp.tile([128, 4 * F1], F32, tag="st1", name="st1")
                            for q in range(4):
                                qq = g * 4 + q
                                sl, osl = bass.ts(qq, 128), bass.ts(q, F1)
                                nc.tensor.matmul(pa[:, osl], lhsT=D[:, sl],
                                                 rhs=cs["W1r65"][0:kp, :],
                                                 start=True, stop=True)
                                nc.tensor.matmul(pai[:, osl], lhsT=D[:, sl],
                                                 rhs=cs["W1i65"][0:kp, :],
                                                 start=True, stop=True)
                            Ar = dp.tile([128, 4 * F1], cdt, tag="ArA", name="ArA")
                            Ai = dp.tile([128, 4 * F1], cdt, tag="AiA", name="AiA")
                            nc.scalar.copy(Ar[:], pa[:])
                            nc.scalar.copy(Ai[:], pai[:])
                            _cmul_psum(nc, dp, nc.vector,
                                       Bs[inp + "r"][:, gsl], Bs[inp + "i"][:, gsl],
                                       Ar[:], Ai[:],
                                       _b3(cs["twAr"][:], 4, F1),
                                       _b3(cs["twAi"][:], 4, F1), F1, 4,
                                       tag="twtA", tdt=cdt)

                    Zs = {k: dp.tile([128, rbb * F1], cdt, tag=f"Z{k}", name=f"Z{k}")
                          for k in ("pr", "pi", "er", "ei")}
                    for g in range(rbb // 4):
                        gsl = bass.ts(g, 4 * F1)
                        for inp in ("p", "e"):
                            br, bi = Bs[inp + "r"], Bs[inp + "i"]
                            pzr = pp.tile([128, 4 * F1], F32, tag="st2", name="st2")
                            pzi = pp.tile([128, 4 * F1], F32, tag="st2", name="st2")
                            nc.tensor.matmul(pzr[:], lhsT=cs["W1r"][:],
                                             rhs=br[:, gsl], start=True, stop=False)
                            nc.tensor.matmul(pzr[:], lhsT=cs["nW1i"][:],
                                             rhs=bi[:, gsl], start=False, stop=True)
                            nc.tensor.matmul(pzi[:], lhsT=cs["W1i"][:],
                                             rhs=br[:, gsl], start=True, stop=False)
                            nc.tensor.matmul(pzi[:], lhsT=cs["W1r"][:],
                                             rhs=bi[:, gsl], start=False, stop=True)
                            nc.scalar.copy(Zs[inp + "r"][:, gsl], pzr[:])
                            nc.scalar.copy(Zs[inp + "i"][:, gsl], pzi[:])

                    Sr = dp.tile([128, rbb * F1], cdt, tag="Sr", name="Sr")
                    Si = dp.tile([128, rbb * F1], cdt, tag="Si", name="Si")
                    tmpb = dp.tile([128, rbb * F1], F32, tag="tmpbig", name="tmpbig")
                    _spectral(nc, nc.gpsimd, Sr[:], Si[:],
                              Zs["pr"][:], Zs["pi"][:], Zs["er"][:], Zs["ei"][:],
                              tmpb[:])

                    Hr = dp.tile([F1, rbb * 128], cdt, tag="Hr", name="Hr")
                    Hi = dp.tile([F1, rbb * 128], cdt, tag="Hi", name="Hi")
                    for g in range(rbb // 4):
                        gsl = bass.ts(g, 512)
                        pgr = pp.tile([F1, 512], F32, tag="inv", name="inv")
                        pgi = pp.tile([F1, 512], F32, tag="inv", name="inv")
                        for q in range(4):
                            qq = g * 4 + q
                            sl, osl = bass.ts(qq, F1), bass.ts(q, 128)
                            nc.tensor.matmul(pgr[:, osl], lhsT=Sr[:, sl],
                                             rhs=cs["V2Ar"][:], start=True, stop=False)
                            nc.tensor.matmul(pgr[:, osl], lhsT=Si[:, sl],
                                             rhs=cs["nV2Ai"][:], start=False, stop=True)
                            nc.tensor.matmul(pgi[:, osl], lhsT=Sr[:, sl],
                                             rhs=cs["V2Ai"][:], start=True, stop=False)
                            nc.tensor.matmul(pgi[:, osl], lhsT=Si[:, sl],
                                             rhs=cs["V2Ar"][:], start=False, stop=True)
                        Gr = dp.tile([F1, 512], cdt, tag="GrA", name="GrA")
                        Gi = dp.tile([F1, 512], cdt, tag="GiA", name="GiA")
                        nc.scalar.copy(Gr[:], pgr[:])
                        nc.scalar.copy(Gi[:], pgi[:])
                        _cmul_psum(nc, dp, nc.vector, Hr[:, gsl], Hi[:, gsl],
                                   Gr[:], Gi[:],
                                   _b3(cs["itwAr"][:], 4, 128),
                                   _b3(cs["itwAi"][:], 4, 128), 128, 4,
                                   part=F1, tag="twtiA", tdt=cdt)

                    convSB = dp.tile([128, rbb * 128], cdt, tag="convSB", name="convSB")
                    for g in range(rbb // 4):
                        gsl = bass.ts(g, 512)
                        pc = pp.tile([128, 512], F32, tag="cc", name="cc")
                        nc.tensor.matmul(pc[:], lhsT=cs["V1Ar"][:],
                                         rhs=Hr[:, gsl], start=True, stop=False)
                        nc.tensor.matmul(pc[:], lhsT=cs["nV1Ai"][:],
                                         rhs=Hi[:, gsl], start=False, stop=True)
                        nc.scalar.copy(convSB[:, gsl], pc[:])

                    nc.sync.dma_start(
                        scratch[r0:r0 + rbb, 0:14336].rearrange(
                            "q (a b) -> q a b", a=112).transpose([1, 0, 2]),
                        convSB[0:112, :].rearrange("p (q b) -> p q b", b=128))
                    nc.sync.dma_start(
                        scratch[r0:r0 + rbb, 14336:14337].rearrange("q x -> x q"),
                        convSB[112:113, 0:rbb * 128:128])
                    nc.sync.dma_start(
                        scratch[r0:r0 + rbb, 14337:14593].rearrange(
                            "q (a b) -> q a b", a=2).transpose([1, 0, 2]),
                        convSB[0:2, :].rearrange("p (q b) -> p q b", b=128))

            # ---------------- C) 32K level ----------------
            with tc.tile_pool(name="p32", bufs=2) as dp:
                for b in range(nb2):
                    r0 = b * rb2
                    D2c = dp.tile([57, rb2 * 256], cdt, tag="D2c", name="D2c")
                    D2t = dp.tile([57, rb2 * 256], cdt, tag="D2t", name="D2t")
                    if cdt == F32:
                        tgt_ = D2t
                    else:
                        tgt_ = dp.tile([57, rb2 * 256], F32, tag="D2ts", name="D2ts")
                    nc.scalar.memzero(D2c[:])
                    nc.scalar.memzero(tgt_[:])
                    nc.sync.dma_start(
                        D2c[0:56, :].rearrange("p (q b) -> p q b", b=256),
                        scratch[r0:r0 + rb2, 0:14336].rearrange(
                            "q (a b) -> q a b", a=56).transpose([1, 0, 2]))
                    nc.sync.dma_start(
                        D2c[56:57, 0:rb2 * 256:256],
                        scratch[r0:r0 + rb2, 14336:14337].rearrange("q x -> x q"))
                    nc.sync.dma_start(
                        tgt_[0:56, :].rearrange("p (q b) -> p q b", b=256),
                        target[r0:r0 + rb2, 0:14336].rearrange(
                            "q (a b) -> q a b", a=56).transpose([1, 0, 2]))
                    nc.sync.dma_start(
                        tgt_[56:57, 0:rb2 * 256:256],
                        target[r0:r0 + rb2, 14336:14337].rearrange("q x -> x q"))
                    if cdt != F32:
                        nc.gpsimd.tensor_copy(D2t[:], tgt_[:])

                    B2 = {}
                    for c in range(2):
                        for inp, D in (("c", D2c), ("t", D2t)):
                            br = dp.tile([128, rb2 * F1], cdt, tag=f"B2r{c}{inp}", name=f"B2r{c}{inp}")
                            bi = dp.tile([128, rb2 * F1], cdt, tag=f"B2i{c}{inp}", name=f"B2i{c}{inp}")
                            pa = pp.tile([128, rb2 * F1], F32, tag="st1", name="st1")
                            pai = pp.tile([128, rb2 * F1], F32, tag="st1", name="st1")
                            for q in range(rb2):
                                dsl = slice(q * 256 + c * 128,
                                            q * 256 + c * 128 + 128)
                                osl = bass.ts(q, F1)
                                nc.tensor.matmul(pa[:, osl], lhsT=D[:, dsl],
                                                 rhs=cs["W1r65"][0:57, :],
                                                 start=True, stop=True)
                                nc.tensor.matmul(pai[:, osl], lhsT=D[:, dsl],
                                                 rhs=cs["W1i65"][0:57, :],
                                                 start=True, stop=True)
                            Ar = dp.tile([128, rb2 * F1], cdt, tag="ArB", name="ArB")
                            Ai = dp.tile([128, rb2 * F1], cdt, tag="AiB", name="AiB")
                            nc.scalar.copy(Ar[:], pa[:])
                            nc.scalar.copy(Ai[:], pai[:])
                            _cmul_psum(nc, dp, nc.vector, br[:], bi[:],
                                       Ar[:], Ai[:],
                                       _b3(cs[f"twBr{c}"][:], rb2, F1),
                                       _b3(cs[f"twBi{c}"][:], rb2, F1), F1, rb2,
                                       tag="twtB", tdt=cdt)
                            B2[(c, inp)] = (br, bi)

                    Z2 = {}
                    for inp in ("c", "t"):
                        for f2c in range(2):
                            zr = dp.tile([128, rb2 * F1], cdt, tag=f"Z2r{inp}{f2c}", name=f"Z2r{inp}{f2c}")
                            zi = dp.tile([128, rb2 * F1], cdt, tag=f"Z2i{inp}{f2c}", name=f"Z2i{inp}{f2c}")
                            pzr = pp.tile([128, rb2 * F1], F32, tag="st2", name="st2")
                            pzi = pp.tile([128, rb2 * F1], F32, tag="st2", name="st2")
                            for t2c in range(2):
                                br, bi = B2[(t2c, inp)]
                                nc.tensor.matmul(pzr[:], lhsT=cs[f"W2Br{t2c}{f2c}"][:],
                                                 rhs=br[:], start=(t2c == 0), stop=False)
                                nc.tensor.matmul(pzr[:], lhsT=cs[f"nW2Bi{t2c}{f2c}"][:],
                                                 rhs=bi[:], start=False, stop=(t2c == 1))
                                nc.tensor.matmul(pzi[:], lhsT=cs[f"W2Bi{t2c}{f2c}"][:],
                                                 rhs=br[:], start=(t2c == 0), stop=False)
                                nc.tensor.matmul(pzi[:], lhsT=cs[f"W2Br{t2c}{f2c}"][:],
                                                 rhs=bi[:], start=False, stop=(t2c == 1))
                            nc.scalar.copy(zr[:], pzr[:])
                            nc.scalar.copy(zi[:], pzi[:])
                            Z2[(inp, f2c)] = (zr, zi)

                    S2 = {}
                    tmpc = dp.tile([128, rb2 * F1], F32, tag="tmpc", name="tmpc")
                    for f2c in range(2):
                        zcr, zci = Z2[("c", f2c)]
                        ztr, zti = Z2[("t", f2c)]
                        sr = dp.tile([128, rb2 * F1], cdt, tag=f"S2r{f2c}", name=f"S2r{f2c}")
                        si = dp.tile([128, rb2 * F1], cdt, tag=f"S2i{f2c}", name=f"S2i{f2c}")
                        _spectral(nc, nc.gpsimd, sr[:], si[:], zcr[:], zci[:],
                                  ztr[:], zti[:], tmpc[:])
                        S2[f2c] = (sr, si)

                    H2r = dp.tile([F1, rb2 * 256], cdt, tag="H2r", name="H2r")
                    H2i = dp.tile([F1, rb2 * 256], cdt, tag="H2i", name="H2i")
                    for g in range(rb2 // 2):
                        pgr = pp.tile([F1, 512], F32, tag="inv", name="inv")
                        pgi = pp.tile([F1, 512], F32, tag="inv", name="inv")
                        for q in range(2):
                            qq = g * 2 + q
                            sl, osl = bass.ts(qq, F1), bass.ts(q, 256)
                            for f2c in range(2):
                                sr, si = S2[f2c]
                                nc.tensor.matmul(pgr[:, osl], lhsT=sr[:, sl],
                                                 rhs=cs[f"V2Br{f2c}"][:],
                                                 start=(f2c == 0), stop=False)
                                nc.tensor.matmul(pgr[:, osl], lhsT=si[:, sl],
                                                 rhs=cs[f"nV2Bi{f2c}"][:],
                                                 start=False, stop=(f2c == 1))
                                nc.tensor.matmul(pgi[:, osl], lhsT=sr[:, sl],
                                                 rhs=cs[f"V2Bi{f2c}"][:],
                                                 start=(f2c == 0), stop=False)
                                nc.tensor.matmul(pgi[:, osl], lhsT=si[:, sl],
                                                 rhs=cs[f"V2Br{f2c}"][:],
                                                 start=False, stop=(f2c == 1))
                        gsl = bass.ts(g, 512)
                        G2r = dp.tile([F1, 512], cdt, tag="G2r", name="G2r")
                        G2i = dp.tile([F1, 512], cdt, tag="G2i", name="G2i")
                        nc.scalar.copy(G2r[:], pgr[:])
                        nc.scalar.copy(G2i[:], pgi[:])
                        _cmul_psum(nc, dp, nc.vector, H2r[:, gsl], H2i[:, gsl],
                                   G2r[:], G2i[:],
                                   _b3(cs["itwBr"][:], 2, 256),
                                   _b3(cs["itwBi"][:], 2, 256), 256, 2,
                                   part=F1, tag="twtiB", tdt=cdt)

                    for g in range(rb2 // 2):
                        gsl = bass.ts(g, 512)
                        pcc = pp.tile([128, 512], F32, tag="cc", name="cc")
                        nc.tensor.matmul(pcc[:], lhsT=cs["V1Br"][:],
                                         rhs=H2r[:, gsl], start=True, stop=False)
                        nc.tensor.matmul(pcc[:], lhsT=cs["nV1Bi"][:],
                                         rhs=H2i[:, gsl], start=False, stop=True)
                        csl = slice((r0 + g * 2) * 256, (r0 + g * 2 + 2) * 256)
                        ccv = ccm_all[:, csl].rearrange("p (a b) -> p a b", b=256)
                        nc.vector.scalar_tensor_tensor(
                            ccv, pcc[:].rearrange("p (a b) -> p a b", b=256),
                            1.0, _b3(cs["maskB"][:], 2, 256),
                            op0=AT.bypass, op1=AT.add)
                        nc.vector.tensor_reduce(
                            allmax[:, r0 + g * 2:r0 + g * 2 + 2], ccv,
                            axis=AX.X, op=AT.max)

            # ---------------- D) argmax -> shifts ----------------
            with tc.tile_pool(name="amax", bufs=1) as dp:
                pt = pp.tile([rows, 128], F32, tag="st1", name="st1")
                nc.tensor.transpose(pt[:], allmax[:, 0:rows], cs["ident"][:])
                tmax = dp.tile([rows, 128], F32, tag="tmax", name="tmax")
                nc.scalar.copy(tmax[:], pt[:])
                rowmax = dp.tile([rows, 1], F32, tag="rowmax", name="rowmax")
                nc.vector.tensor_reduce(rowmax[:], tmax[:], axis=AX.X, op=AT.max)
                prm = pp.tile([1, rows], F32, tag="st2", name="st2")
                nc.tensor.transpose(prm[:], rowmax[:], cs["ident"][0:rows, 0:rows])
                rmT = dp.tile([1, rows], F32, tag="rmT", name="rmT")
                nc.scalar.copy(rmT[:], prm[:])
                pmb = pp.tile([128, rows], F32, tag="inv", name="inv")
                nc.tensor.matmul(pmb[:], lhsT=cs["ones1x128"][:], rhs=rmT[:],
                                 start=True, stop=True)
                Mb = dp.tile([128, rows], BF16, tag="Mb", name="Mb")
                nc.scalar.copy(Mb[:], pmb[:])

                eqm = dp.tile([128, min(rows, 8) * 256], BF16, tag="eqm", name="eqm")
                selm = dp.tile([128, min(rows, 8) * 256], F32, tag="selm", name="selm")
                for bb in range(max(1, rows // 8)):
                    csl = bass.ts(bb, min(rows, 8) * 256)
                    nr8 = min(rows, 8)
                    mbb = Mb[:, bb * nr8:(bb + 1) * nr8]\
                        .rearrange("p (a b) -> p a b", b=1)\
                        .to_broadcast([128, nr8, 256])
                    ccv = ccm_all[:, csl].rearrange("p (a b) -> p a b", b=256)
                    nc.vector.tensor_tensor(
                        eqm[:].rearrange("p (a b) -> p a b", b=256),
                        ccv, mbb, op=AT.is_equal)
                    nc.vector.tensor_tensor(
                        selm[:].rearrange("p (a b) -> p a b", b=256),
                        eqm[:].rearrange("p (a b) -> p a b", b=256),
                        _b3(cs["shvB"][:], nr8, 256), op=AT.mult)
                    nc.vector.tensor_reduce(
                        allmin[:, bb * nr8:(bb + 1) * nr8],
                        selm[:].rearrange("p (a b) -> p a b", b=256),
                        axis=AX.X, op=AT.min)
                pt2 = pp.tile([rows, 128], F32, tag="cc", name="cc")
                nc.tensor.transpose(pt2[:], allmin[:, 0:rows], cs["ident"][:])
                tmin = dp.tile([rows, 128], F32, tag="tmin", name="tmin")
                nc.scalar.copy(tmin[:], pt2[:])
                nc.vector.tensor_reduce(shifts[:], tmin[:], axis=AX.X, op=AT.min)
                nc.vector.tensor_scalar_add(shifts[:], shifts[:], BIGL + float(START0))

                # start = (7040 + shift) mod 14337
                m1 = dp.tile([rows, 1], F32, tag="m1", name="m1")
                nc.vector.tensor_scalar(out=m1[:], in0=shifts[:], scalar1=0.0,
                                        scalar2=None, op0=AT.is_lt)
                nc.vector.scalar_tensor_tensor(
                    shifts[:], m1[:], float(CONV_LEN), shifts[:],
                    op0=AT.mult, op1=AT.add)
                nc.vector.tensor_scalar(out=m1[:], in0=shifts[:],
                                        scalar1=float(CONV_LEN), scalar2=None,
                                        op0=AT.is_ge)
                nc.vector.scalar_tensor_tensor(
                    shifts[:], m1[:], float(-CONV_LEN), shifts[:],
                    op0=AT.mult, op1=AT.add)

                idxf = dp.tile([rows, CROP], F32, tag="idxf", name="idxf")
                nc.vector.tensor_tensor(idxf[:], cs["winidx"][0:rows, :],
                                        shifts[:].to_broadcast([rows, CROP]),
                                        op=AT.add)
                idxi = dp.tile([rows, CROP], I32, tag="idxi", name="idxi")
                nc.vector.tensor_copy(idxi[:], idxf[:])
                w = dp.tile([rows, CROP], cdt, tag="wg", name="wg")
                nc.gpsimd.indirect_dma_start(
                    out=w[:], out_offset=None,
                    in_=scratch.ap().rearrange("r p -> (r p)").rearrange(
                        "(a b) -> a b", b=1),
                    in_offset=bass.IndirectOffsetOnAxis(ap=idxi[:], axis=0),
                )
                tw_ = dp.tile([rows, CROP], F32, tag="twin", name="twin")
                nc.sync.dma_start(tw_[:], target[:, START0:START0 + CROP])
                nc.vector.tensor_tensor(w[:], w[:], tw_[:], op=AT.subtract)
                convacc = dp.tile([rows, 1], F32, tag="convacc", name="convacc")
                nc.vector.scalar_tensor_tensor(
                    tw_[:], w[:], 1.0, w[:], op0=AT.bypass, op1=AT.mult,
                    accum_out=convacc[:])

                a0 = dp.tile([128, 1], F32, tag="a0", name="a0")
                nc.vector.tensor_reduce(a0[:], astf_acc[:], axis=AX.X, op=AT.add)
                psa = pp.tile([1, 1], F32, tag="st1", name="st1")
                nc.tensor.matmul(psa[:], lhsT=a0[:], rhs=cs["ones128"][:],
                                 start=True, stop=True)
                psc = pp.tile([1, 1], F32, tag="st2", name="st2")
                nc.tensor.matmul(psc[:], lhsT=convacc[:], rhs=cs["ones64"][0:rows, :],
                                 start=True, stop=True)
                nc.scalar.copy(outt[:, 0:1], psa[:])
                nc.scalar.copy(outt[:, 1:2], psc[:])
                nc.sync.dma_start(out[:], outt[:])

    nc.finalize()
    return nc, consts


_CACHE = {}


def get_built(cdt=BF16):
    key = str(cdt)
    if key not in _CACHE:
        _CACHE[key] = build_nc(cdt=cdt)
    return _CACHE[key]


LAST_RESULT = {}


def kernel(pred_astf, true_astf, egf, target_waveform):
    import os
    from concourse.bass_utils import run_bass_kernel_spmd
    cdt = F32 if os.environ.get("CONVALIGN_F32") == "1" else BF16
    nc, consts = get_built(cdt)
    if cdt != F32:
        import ml_dtypes
        mmnames = _mm_const_names()
        consts = {k: (v.astype(ml_dtypes.bfloat16) if k in mmnames else v)
                  for k, v in consts.items()}
    pred_astf = np.ascontiguousarray(np.asarray(pred_astf, np.float32))
    true_astf = np.ascontiguousarray(np.asarray(true_astf, np.float32))
    egf = np.ascontiguousarray(np.asarray(egf, np.float32))
    target_waveform = np.ascontiguousarray(
        np.asarray(target_waveform, np.float32))
    B = pred_astf.shape[0]
    per = B // NCORES
    in_maps = []
    for i in range(NCORES):
        sl = slice(i * per, (i + 1) * per)
        m = {"pred": pred_astf[sl], "true": true_astf[sl],
             "egf": egf[sl], "target": target_waveform[sl]}
        m.update(consts)
        in_maps.append(m)
    trace = os.environ.get("CONVALIGN_TRACE") == "1"
    res = run_bass_kernel_spmd(nc, in_maps, core_ids=list(range(NCORES)),
                               trace=trace)
    LAST_RESULT["res"] = res
    sums = np.stack([res.results[i]["out"][0] for i in range(NCORES)])
    loss_astf = np.float32(sums[:, 0].sum() / (B * L1))
    loss_conv = np.float32(sums[:, 1].sum() / (B * CROP))
    total = np.float32(loss_astf + loss_conv)
    return total, loss_astf, loss_conv



# revision 10
# speedup vs baseline: 1.2273x; 1.2273x over previous
"""Trainium2 Bass kernel for nn_ConvAlignLoss (8-core data parallel), v2.

Self-contained: hardcodes shapes; imports concourse from /opt/trn_rl_repo.

Per core (R=64 rows):
  loss_astf partial: sum((pred-true)^2)   (folded into the 16K block loop)
  conv = irfft16384(fft(pred) * conj(fft(egf_pad)))[:14337]  (2-stage matmul FFT)
  cc   = irfft32768(fft(conv_pad) * conj(fft(target_pad)))
  shift = argmax of cc == argmin of shift-encoded value at row max
  loss_conv partial: sum((conv[(7040+i+shift) % 14337] - target[7040+i])^2)
Host combines the 8 cores' (sum_astf, sum_conv) into the scalar losses.

v2 structural changes vs v1 (713us):
  - stage-1 matmuls emit (re|im) in one pass via stacked rhs [W1r|W1i]
  - inverse G-stage emits (Gr|Gi) in one psum via stacked rhs [V2r|V2i]
  - twiddle cmuls batched to full-block strided TTs (bf16 2x_1p mode)
  - input casts moved off Pool onto ACT
  - per-(t1,row) argmax candidates computed inline in the 32K loop; the
    global argmax tail is tiny
  - scratch writes issued from Pool so the 32K loads (sync) prefetch early
  - consts packed into 4 stacked dram tensors (few DMAs, sliced in SBUF)
"""
import sys

sys.path.insert(0, "/opt/trn_rl_repo")

import numpy as np
import concourse.bass as bass
import concourse.bacc as bacc
import concourse.mybir as mybir
from concourse import tile

F32 = mybir.dt.float32
BF16 = mybir.dt.bfloat16
I32 = mybir.dt.int32
AT = mybir.AluOpType
AX = mybir.AxisListType

R = 64
NCORES = 8
L1, L2 = 16384, 2048
CONV_LEN = L1 - L2 + 1      # 14337
N_A, N_B = 16384, 32768
GAP_LO, GAP_HI = CONV_LEN, N_B - CONV_LEN + 1   # gap [14337, 18432)
CROP = 256
START0 = (CONV_LEN - CROP) // 2                 # 7040
PITCH = 14720
BIGL = float(2 ** 23)
F1 = 65                      # Hermitian half: f1 in [0, 64]


def _dft(n, sign):
    k = np.arange(n)
    return np.exp(sign * 2j * np.pi * np.outer(k, k) / n)


def make_packed_consts():
    """Build the packed const arrays + per-name (group, offset, width)."""
    cf = np.ones(F1)
    cf[1:64] = 2.0

    W1 = _dft(128, -1)
    V2A = _dft(128, +1)
    twA = np.exp(-2j * np.pi * np.outer(np.arange(128), np.arange(F1)) / N_A)
    itwA = np.exp(2j * np.pi * np.outer(np.arange(F1), np.arange(128)) / N_A)
    V1A = cf[:, None] * _dft(128, +1)[:F1] / N_A        # [f1<=64, t1]

    W2B = _dft(256, -1)          # [t2, f2]
    twB = np.exp(-2j * np.pi * np.outer(np.arange(256), np.arange(F1)) / N_B)
    V2B = _dft(256, +1)          # [f2, t2]
    itwB = np.exp(2j * np.pi * np.outer(np.arange(F1), np.arange(256)) / N_B)
    V1B = cf[:, None] * _dft(128, +1)[:F1] / N_B        # [f1<=64, t1]

    # twB reshaped: [p=128, c=2, f=65] -> [128, 130], then tiled x2 (inp)
    twB2r = np.stack([twB.real[0:128], twB.real[128:256]], axis=1).reshape(128, 130)
    twB2i = np.stack([twB.imag[0:128], twB.imag[128:256]], axis=1).reshape(128, 130)
    twB4r = np.hstack([twB2r, twB2r])
    twB4i = np.hstack([twB2i, twB2i])

    j = np.arange(128)[:, None] * 256 + np.arange(256)[None, :]   # [t1, t2]
    gap = (j >= GAP_LO) & (j < GAP_HI)
    maskB = np.where(gap, -1e30, 0.0)
    shiftval = np.where(j <= CONV_LEN - 1, j - (CONV_LEN - 1), j - GAP_HI + 1)
    shvB = np.where(gap, 0.0, shiftval - BIGL)
    winidx = (np.arange(R)[:, None] * PITCH
              + np.arange(CROP)[None, :])                         # [64, 256]

    groups = {}   # gname -> list of (name, arr)

    def put(g, name, arr):
        groups.setdefault(g, []).append(
            (name, np.ascontiguousarray(np.asarray(arr, np.float64))))

    # ---- [128, X] bf16 group ----
    put("cb128", "W1ri65", np.hstack([W1.real[:, :F1], W1.imag[:, :F1]]))
    put("cb128", "W1r", W1.real)      # row 0 is all-ones (used as ones-row)
    put("cb128", "nW1i", -W1.imag)
    put("cb128", "W1i", W1.imag)
    put("cb128", "twAr", twA.real)
    put("cb128", "twAi", twA.imag)
    put("cb128", "V2A_rI", np.hstack([V2A.real, V2A.imag]))
    put("cb128", "V2A_iR", np.hstack([-V2A.imag, V2A.real]))
    for a in range(2):
        for b in range(2):
            blk = W2B[a * 128:(a + 1) * 128, b * 128:(b + 1) * 128]
            put("cb128", f"W2Br{a}{b}", blk.real)
            put("cb128", f"W2Bi{a}{b}", blk.imag)
            put("cb128", f"nW2Bi{a}{b}", -blk.imag)
    put("cb128", "twB4r", twB4r)
    put("cb128", "twB4i", twB4i)
    for a in range(2):
        blk = V2B[a * 128:(a + 1) * 128, :]
        put("cb128", f"V2B_rI{a}", np.hstack([blk.real, blk.imag]))
        put("cb128", f"V2B_iR{a}", np.hstack([-blk.imag, blk.real]))
    put("cb128", "ident", np.eye(128))
    # ---- [65, X] bf16 group ----
    put("cb65", "itwAr", itwA.real)
    put("cb65", "itwAi", itwA.imag)
    put("cb65", "V1Ar", V1A.real)
    put("cb65", "nV1Ai", -V1A.imag)
    put("cb65", "itwBr", itwB.real)
    put("cb65", "itwBi", itwB.imag)
    put("cb65", "V1Br", V1B.real)
    put("cb65", "nV1Bi", -V1B.imag)
    # ---- [128, X] f32 group ----
    put("cf128", "maskB", maskB)
    put("cf128", "shvB", shvB)
    put("cf128", "identF", np.eye(128))
    put("cf128", "ones128", np.ones((128, 1)))
    # ---- [64, X] f32 group ----
    put("cf64", "winidx", winidx)
    put("cf64", "ones64", np.ones((R, 1)))

    gdtype = {"cb128": "bf16", "cb65": "bf16", "cf128": "f32", "cf64": "f32"}
    packed = {}
    layout = {}
    for g, items in groups.items():
        parts = []
        off = 0
        for name, arr in items:
            assert arr.ndim == 2
            layout[name] = (g, off, arr.shape[1])
            parts.append(arr)
            off += arr.shape[1]
        cat = np.hstack(parts)
        if gdtype[g] == "f32":
            packed[g] = cat.astype(np.float32)
        else:
            import ml_dtypes
            packed[g] = cat.astype(ml_dtypes.bfloat16)
    return packed, layout


def build_nc():
    nc = bacc.Bacc("TRN2", target_bir_lowering=False, debug=False,
                   num_devices=NCORES)
    packed, layout = make_packed_consts()

    pred = nc.dram_tensor("pred", [R, L1], F32, kind="ExternalInput")
    true_ = nc.dram_tensor("true", [R, L1], F32, kind="ExternalInput")
    egf = nc.dram_tensor("egf", [R, L2], F32, kind="ExternalInput")
    target = nc.dram_tensor("target", [R, CONV_LEN], F32,
                            kind="ExternalInput")
    out = nc.dram_tensor("out", [1, 2], F32, kind="ExternalOutput")
    scratch = nc.dram_tensor("scratch", [R, PITCH], BF16)

    cdram = {}
    for g, arr in packed.items():
        dt = F32 if arr.dtype == np.float32 else BF16
        cdram[g] = nc.dram_tensor(g, list(arr.shape), dt, kind="ExternalInput")

    NB1 = R // 8    # 8 blocks of 8 rows (16K level)
    NB2 = R // 4    # 16 blocks of 4 rows (32K level)

    def _cp(eng, out_ap, in_ap):
        if eng is nc.scalar:
            nc.scalar.copy(out_ap, in_ap)
        else:
            eng.tensor_copy(out_ap, in_ap)

    with tile.TileContext(nc) as tc:
        with (
            tc.tile_pool(name="consts", bufs=1) as cpool,
            tc.tile_pool(name="keep", bufs=1) as kpool,
        ):
            ctile = {}
            for g, arr in packed.items():
                dt = F32 if arr.dtype == np.float32 else BF16
                t = cpool.tile(list(arr.shape), dt, tag=f"c_{g}", name=f"c_{g}")
                nc.sync.dma_start(t[:], cdram[g][:])
                ctile[g] = t

            def cs(name):
                g, off, w = layout[name]
                return ctile[g][:, off:off + w]

            astf_acc = kpool.tile([128, NB1], F32, tag="astfacc", name="astfacc")
            allmax = kpool.tile([128, R], BF16, tag="allmax", name="allmax")
            argshv = kpool.tile([128, R], F32, tag="argshv", name="argshv")
            shifts = kpool.tile([R, 1], F32, tag="shifts", name="shifts")
            outt = kpool.tile([1, 2], F32, tag="outt", name="outt")

            # egf loaded once: [16, R*128] f32 -> bf16
            egf_f = kpool.tile([16, R * 128], F32, tag="egff", name="egff")
            egf_b = kpool.tile([16, R * 128], BF16, tag="egfb", name="egfb")
            nc.sync.dma_start(
                egf_f[:].rearrange("p (r b) -> p r b", b=128),
                egf.ap().rearrange("r (a b) -> a r b", a=16))
            nc.scalar.copy(egf_b[:], egf_f[:])

            # PSUM pools: 6 banks double-buffered + 2 single (8 total)
            _ps2ctx = tc.tile_pool(name="ps2", bufs=2, space="PSUM")
            pp = _ps2ctx.__enter__()
            _ps1ctx = tc.tile_pool(name="ps1", bufs=1, space="PSUM")
            pq = _ps1ctx.__enter__()

            # ---------------- B) 16K level + astf ----------------
            with tc.tile_pool(name="p16", bufs=2) as dp:
                for b in range(NB1):
                    r0 = b * 8
                    Dp = dp.tile([128, 1024], F32, tag="Dp", name="Dp")
                    Dt = dp.tile([128, 1024], F32, tag="Dt", name="Dt")
                    Dpb = dp.tile([128, 1024], BF16, tag="Dpb", name="Dpb")
                    psrc = pred[r0:r0 + 8, :].rearrange(
                        "q (a b) -> q a b", a=128).transpose([1, 0, 2])
                    tsrc = true_[r0:r0 + 8, :].rearrange(
                        "q (a b) -> q a b", a=128).transpose([1, 0, 2])
                    nc.sync.dma_start(
                        Dp[:].rearrange("p (q b) -> p q b", b=128), psrc)
                    nc.sync.dma_start(
                        Dt[:].rearrange("p (q b) -> p q b", b=128), tsrc)
                    # astf: Dt = Dp - Dt (Pool); accum (Dt*Dt) (DVE)
                    nc.gpsimd.tensor_tensor(Dt[:], Dp[:], Dt[:], op=AT.subtract)
                    nc.vector.scalar_tensor_tensor(
                        Dt[:], Dt[:], 1.0, Dt[:], op0=AT.bypass, op1=AT.mult,
                        accum_out=astf_acc[:, b:b + 1])
                    nc.scalar.copy(Dpb[:], Dp[:])   # cast f32->bf16 (ACT)

                    # --- s1: A[t2, (inp,q,ri,f1)], inp-major ---
                    A = dp.tile([128, 2080], BF16, tag="A16", name="A16")
                    cp_eng = [nc.scalar, nc.scalar, nc.vector, nc.scalar,
                              nc.scalar, nc.vector, nc.scalar, nc.scalar]
                    for qp in range(4):
                        ps = pp.tile([128, 260], F32, tag="s1", name="s1")
                        for h in range(2):
                            q = qp * 2 + h
                            nc.tensor.matmul(
                                ps[:, h * 130:(h + 1) * 130],
                                lhsT=Dpb[:, q * 128:(q + 1) * 128],
                                rhs=cs("W1ri65"), start=True, stop=True)
                        _cp(cp_eng[qp], A[:, qp * 260:(qp + 1) * 260], ps[:])
                    for qp in range(4):
                        ps = pp.tile([128, 260], F32, tag="s1", name="s1")
                        for h in range(2):
                            q = qp * 2 + h
                            gq = r0 + q
                            nc.tensor.matmul(
                                ps[:, h * 130:(h + 1) * 130],
                                lhsT=egf_b[:, gq * 128:(gq + 1) * 128],
                                rhs=cs("W1ri65")[0:16, :], start=True, stop=True)
                        _cp(cp_eng[4 + qp],
                            A[:, 1040 + qp * 260:1040 + (qp + 1) * 260], ps[:])

                    # --- fwd twiddle cmul: B = A * twA  (16 (inp,q) units) ---
                    B = dp.tile([128, 2080], BF16, tag="B16", name="B16")
                    tmpV = dp.tile([128, 1040], BF16, tag="tmpV", name="tmpV")
                    tmpP = dp.tile([128, 1040], BF16, tag="tmpP", name="tmpP")

                    def v16(t, lo, hi):
                        return t[:].rearrange("p (a x) -> p a x", a=16)[:, :, lo:hi]

                    twr = cs("twAr").rearrange("p (a x) -> p a x", a=1)\
                        .to_broadcast([128, 16, F1])
                    twi = cs("twAi").rearrange("p (a x) -> p a x", a=1)\
                        .to_broadcast([128, 16, F1])
                    ar, ai = v16(A, 0, 65), v16(A, 65, 130)
                    br, bi = v16(B, 0, 65), v16(B, 65, 130)
                    tv = tmpV[:].rearrange("p (a x) -> p a x", a=16)[:, :, 0:65]
                    tp = tmpP[:].rearrange("p (a x) -> p a x", a=16)[:, :, 0:65]
                    nc.vector.tensor_tensor(br, ar, twr, op=AT.mult)
                    nc.vector.tensor_tensor(tv, ai, twi, op=AT.mult)
                    nc.vector.tensor_tensor(br, br, tv, op=AT.subtract)
                    nc.gpsimd.tensor_tensor(bi, ar, twi, op=AT.mult)
                    nc.vector.tensor_tensor(tp, ai, twr, op=AT.mult)
                    nc.vector.tensor_tensor(bi, bi, tp, op=AT.add)

                    # --- s2: Z[f2, (inp,g,ri,260)] ---
                    Z = dp.tile([128, 2340], BF16, tag="Z16", name="Z16")
                    zc_eng = [nc.scalar, nc.vector, nc.scalar, nc.vector,
                              nc.scalar, nc.scalar, nc.scalar, nc.scalar]
                    zi_ = 0
                    for inp in range(2):
                        for g in range(2):
                            base = inp * 1040 + g * 520
                            rvv = B[:, base:base + 520].rearrange(
                                "p (a x) -> p a x", a=4)
                            pzr = pq.tile([128, 260], F32, tag="s2r", name="s2r")
                            pzi = pq.tile([128, 260], F32, tag="s2i", name="s2i")
                            nc.tensor.matmul(pzr[:], lhsT=cs("W1r"),
                                             rhs=rvv[:, :, 0:65],
                                             start=True, stop=False)
                            nc.tensor.matmul(pzr[:], lhsT=cs("nW1i"),
                                             rhs=rvv[:, :, 65:130],
                                             start=False, stop=True)
                            nc.tensor.matmul(pzi[:], lhsT=cs("W1i"),
                                             rhs=rvv[:, :, 0:65],
                                             start=True, stop=False)
                            nc.tensor.matmul(pzi[:], lhsT=cs("W1r"),
                                             rhs=rvv[:, :, 65:130],
                                             start=False, stop=True)
                            zb = inp * 1040 + g * 520
                            _cp(zc_eng[zi_], Z[:, zb:zb + 260], pzr[:])
                            _cp(zc_eng[zi_ + 1], Z[:, zb + 260:zb + 520], pzi[:])
                            zi_ += 2

                    # --- spectral: S = Zp * conj(Ze), batched over g ---
                    S = dp.tile([128, 1300], BF16, tag="S16", name="S16")

                    def vz(inp, ri):
                        o = inp * 1040 + ri * 260
                        return Z[:, o:o + 1040].rearrange(
                            "p (a x) -> p a x", a=2)[:, :, 0:260]

                    def vsS(ri):
                        o = ri * 260
                        return S[:, o:o + 1040].rearrange(
                            "p (a x) -> p a x", a=2)[:, :, 0:260]

                    zpr, zpi = vz(0, 0), vz(0, 1)
                    zer, zei = vz(1, 0), vz(1, 1)
                    sr, si = vsS(0), vsS(1)
                    tv2 = tmpV[:, 0:520].rearrange(
                        "p (a x) -> p a x", a=2)
                    tp2 = tmpP[:, 0:520].rearrange(
                        "p (a x) -> p a x", a=2)
                    nc.vector.tensor_tensor(sr, zpr, zer, op=AT.mult)
                    nc.vector.tensor_tensor(tv2, zpi, zei, op=AT.mult)
                    nc.vector.tensor_tensor(sr, sr, tv2, op=AT.add)
                    nc.gpsimd.tensor_tensor(si, zpi, zer, op=AT.mult)
                    nc.vector.tensor_tensor(tp2, zpr, zei, op=AT.mult)
                    nc.vector.tensor_tensor(si, si, tp2, op=AT.subtract)

                    # --- inverse G: psum (Gr|Gi) per q-pair ---
                    Gsb = dp.tile([F1, 2048], BF16, tag="G16", name="G16")
                    gc_eng = [nc.scalar, nc.vector, nc.scalar, nc.scalar]
                    for qp in range(4):
                        psG = pp.tile([F1, 512], F32, tag="G", name="G")
                        for h in range(2):
                            q = qp * 2 + h
                            g, qq = q // 4, q % 4
                            sro = g * 520 + qq * 65
                            sio = 260 + g * 520 + qq * 65
                            nc.tensor.matmul(
                                psG[:, h * 256:(h + 1) * 256],
                                lhsT=S[:, sro:sro + 65], rhs=cs("V2A_rI"),
                                start=True, stop=False)
                            nc.tensor.matmul(
                                psG[:, h * 256:(h + 1) * 256],
                                lhsT=S[:, sio:sio + 65], rhs=cs("V2A_iR"),
                                start=False, stop=True)
                        _cp(gc_eng[qp], Gsb[:, qp * 512:(qp + 1) * 512], psG[:])

                    # --- itw cmul: H = G * itwA  (8 q units of (ri,128)) ---
                    H = dp.tile([F1, 2048], BF16, tag="H16", name="H16")

                    def vg(t, lo, hi):
                        return t[:].rearrange("p (a x) -> p a x", a=8)[:, :, lo:hi]

                    itr = cs("itwAr")[0:F1, :].rearrange(
                        "p (a x) -> p a x", a=1).to_broadcast([F1, 8, 128])
                    iti = cs("itwAi")[0:F1, :].rearrange(
                        "p (a x) -> p a x", a=1).to_broadcast([F1, 8, 128])
                    gr, gi = vg(Gsb, 0, 128), vg(Gsb, 128, 256)
                    hr, hi = vg(H, 0, 128), vg(H, 128, 256)
                    tvh = tmpV[:F1, 0:1024].rearrange("p (a x) -> p a x", a=8)
                    tph = tmpP[:F1, 0:1024].rearrange("p (a x) -> p a x", a=8)
                    nc.vector.tensor_tensor(hr, gr, itr, op=AT.mult)
                    nc.vector.tensor_tensor(tvh, gi, iti, op=AT.mult)
                    nc.vector.tensor_tensor(hr, hr, tvh, op=AT.subtract)
                    nc.gpsimd.tensor_tensor(hi, gr, iti, op=AT.mult)
                    nc.vector.tensor_tensor(tph, gi, itr, op=AT.mult)
                    nc.vector.tensor_tensor(hi, hi, tph, op=AT.add)

                    # --- V1: conv rows ---
                    convSB = dp.tile([128, 1024], BF16, tag="convSB",
                                     name="convSB")
                    for gg in range(2):
                        psX = pp.tile([128, 512], F32, tag="V1", name="V1")
                        hh = H[:, gg * 1024:gg * 1024 + 1024].rearrange(
                            "p (a x) -> p a x", a=4)
                        nc.tensor.matmul(psX[:], lhsT=cs("V1Ar")[0:F1, :],
                                         rhs=hh[:, :, 0:128],
                                         start=True, stop=False)
                        nc.tensor.matmul(psX[:], lhsT=cs("nV1Ai")[0:F1, :],
                                         rhs=hh[:, :, 128:256],
                                         start=False, stop=True)
                        nc.scalar.copy(
                            convSB[:, gg * 512:(gg + 1) * 512], psX[:])

                    # --- scratch writes (Pool-issued so C loads aren't gated)
                    nc.gpsimd.dma_start(
                        scratch[r0:r0 + 8, 0:14336].rearrange(
                            "q (a b) -> q a b", a=112).transpose([1, 0, 2]),
                        convSB[0:112, :].rearrange("p (q b) -> p q b", b=128))
                    nc.gpsimd.dma_start(
                        scratch[r0:r0 + 8, 14336:14337].rearrange("q x -> x q"),
                        convSB[112:113, 0:1024:128])
                    nc.gpsimd.dma_start(
                        scratch[r0:r0 + 8, 14337:14593].rearrange(
                            "q (a b) -> q a b", a=2).transpose([1, 0, 2]),
                        convSB[0:2, :].rearrange("p (q b) -> p q b", b=128))

            # ---------------- C) 32K level ----------------
            with tc.tile_pool(name="p32", bufs=2) as dp:
                for cb in range(NB2):
                    r0 = cb * 4
                    D2c = dp.tile([57, 1024], BF16, tag="D2c", name="D2c")
                    tgtf = dp.tile([57, 1024], F32, tag="tgtf", name="tgtf")
                    D2t = dp.tile([57, 1024], BF16, tag="D2t", name="D2t")
                    nc.scalar.memzero(D2c[:])
                    nc.scalar.memzero(tgtf[:])
                    nc.sync.dma_start(
                        D2c[0:56, :].rearrange("p (q b) -> p q b", b=256),
                        scratch[r0:r0 + 4, 0:14336].rearrange(
                            "q (a b) -> q a b", a=56).transpose([1, 0, 2]))
                    nc.sync.dma_start(
                        D2c[56:57, 0:1024:256],
                        scratch[r0:r0 + 4, 14336:14337].rearrange("q x -> x q"))
                    nc.sync.dma_start(
                        tgtf[0:56, :].rearrange("p (q b) -> p q b", b=256),
                        target[r0:r0 + 4, 0:14336].rearrange(
                            "q (a b) -> q a b", a=56).transpose([1, 0, 2]))
                    nc.sync.dma_start(
                        tgtf[56:57, 0:1024:256],
                        target[r0:r0 + 4, 14336:14337].rearrange("q x -> x q"))
                    nc.scalar.copy(D2t[:], tgtf[:])

                    # --- s1: A2[t2half, (inp,c,q,ri,f1)] ---
                    A2 = dp.tile([128, 2080], BF16, tag="A32", name="A32")
                    c_eng = [nc.scalar, nc.vector, nc.scalar, nc.vector,
                             nc.scalar, nc.scalar, nc.scalar, nc.scalar]
                    ci = 0
                    for inp, D in ((0, D2c), (1, D2t)):
                        for c in range(2):
                            for qp in range(2):
                                ps = pp.tile([128, 260], F32, tag="s1",
                                             name="s1")
                                for h in range(2):
                                    q = qp * 2 + h
                                    nc.tensor.matmul(
                                        ps[:, h * 130:(h + 1) * 130],
                                        lhsT=D[:, q * 256 + c * 128:
                                               q * 256 + c * 128 + 128],
                                        rhs=cs("W1ri65")[0:57, :],
                                        start=True, stop=True)
                                dst0 = inp * 1040 + c * 520 + qp * 260
                                _cp(c_eng[ci % 8], A2[:, dst0:dst0 + 260], ps[:])
                                ci += 1

                    # --- fwd twiddle cmul (per-c twiddle), (inp,c)x(q) ---
                    B2 = dp.tile([128, 2080], BF16, tag="B32", name="B32")
                    tmpV = dp.tile([128, 1040], BF16, tag="tmpV2", name="tmpV2")
                    tmpP = dp.tile([128, 1040], BF16, tag="tmpP2", name="tmpP2")

                    def v32(t, lo, hi):
                        return t[:].rearrange(
                            "p (a q x) -> p a q x", a=4, q=4)[:, :, :, lo:hi]

                    twr = cs("twB4r").rearrange(
                        "p (a q x) -> p a q x", a=4, q=1).to_broadcast(
                        [128, 4, 4, F1])
                    twi = cs("twB4i").rearrange(
                        "p (a q x) -> p a q x", a=4, q=1).to_broadcast(
                        [128, 4, 4, F1])
                    ar, ai = v32(A2, 0, 65), v32(A2, 65, 130)
                    br, bi = v32(B2, 0, 65), v32(B2, 65, 130)
                    tv = tmpV[:].rearrange(
                        "p (a q x) -> p a q x", a=4, q=4)[:, :, :, 0:65]
                    tp = tmpP[:].rearrange(
                        "p (a q x) -> p a q x", a=4, q=4)[:, :, :, 0:65]
                    nc.vector.tensor_tensor(br, ar, twr, op=AT.mult)
                    nc.vector.tensor_tensor(tv, ai, twi, op=AT.mult)
                    nc.vector.tensor_tensor(br, br, tv, op=AT.subtract)
                    nc.gpsimd.tensor_tensor(bi, ar, twi, op=AT.mult)
                    nc.vector.tensor_tensor(tp, ai, twr, op=AT.mult)
                    nc.vector.tensor_tensor(bi, bi, tp, op=AT.add)

                    # --- s2: Z2[f2, (inp,f2c,ri,260)] ---
                    Z2 = dp.tile([128, 2340], BF16, tag="Z32", name="Z32")
                    zi_ = 0
                    for inp in range(2):
                        for f2c in range(2):
                            pzr = pq.tile([128, 260], F32, tag="s2r", name="s2r")
                            pzi = pq.tile([128, 260], F32, tag="s2i", name="s2i")

                            def rv(c, ri):
                                base = inp * 1040 + c * 520
                                return B2[:, base:base + 520].rearrange(
                                    "p (q x) -> p q x", q=4)[
                                    :, :, ri * 65:(ri + 1) * 65]

                            for c in range(2):
                                nc.tensor.matmul(
                                    pzr[:], lhsT=cs(f"W2Br{c}{f2c}"),
                                    rhs=rv(c, 0), start=(c == 0), stop=False)
                                nc.tensor.matmul(
                                    pzr[:], lhsT=cs(f"nW2Bi{c}{f2c}"),
                                    rhs=rv(c, 1), start=False, stop=(c == 1))
                            for c in range(2):
                                nc.tensor.matmul(
                                    pzi[:], lhsT=cs(f"W2Bi{c}{f2c}"),
                                    rhs=rv(c, 0), start=(c == 0), stop=False)
                                nc.tensor.matmul(
                                    pzi[:], lhsT=cs(f"W2Br{c}{f2c}"),
                                    rhs=rv(c, 1), start=False, stop=(c == 1))
                            zb = inp * 1040 + f2c * 520
                            _cp(c_eng[zi_ % 8], Z2[:, zb:zb + 260], pzr[:])
                            _cp(c_eng[(zi_ + 1) % 8],
                                Z2[:, zb + 260:zb + 520], pzi[:])
                            zi_ += 2

                    # --- spectral: S2 = Zc * conj(Zt), batched over f2c ---
                    S2 = dp.tile([128, 1300], BF16, tag="S32", name="S32")

                    def vz2(inp, ri):
                        o = inp * 1040 + ri * 260
                        return Z2[:, o:o + 1040].rearrange(
                            "p (a x) -> p a x", a=2)[:, :, 0:260]

                    def vs2(ri):
                        o = ri * 260
                        return S2[:, o:o + 1040].rearrange(
                            "p (a x) -> p a x", a=2)[:, :, 0:260]

                    zpr, zpi = vz2(0, 0), vz2(0, 1)
                    zer, zei = vz2(1, 0), vz2(1, 1)
                    sr, si = vs2(0), vs2(1)
                    tv2 = tmpV[:, 0:520].rearrange("p (a x) -> p a x", a=2)
                    tp2 = tmpP[:, 0:520].rearrange("p (a x) -> p a x", a=2)
                    nc.vector.tensor_tensor(sr, zpr, zer, op=AT.mult)
                    nc.vector.tensor_tensor(tv2, zpi, zei, op=AT.mult)
                    nc.vector.tensor_tensor(sr, sr, tv2, op=AT.add)
                    nc.gpsimd.tensor_tensor(si, zpi, zer, op=AT.mult)
                    nc.vector.tensor_tensor(tp2, zpr, zei, op=AT.mult)
                    nc.vector.tensor_tensor(si, si, tp2, op=AT.subtract)

                    # --- inverse G2: psum (Gr|Gi) per row ---
                    G2sb = dp.tile([F1, 2048], BF16, tag="G32", name="G32")
                    for q in range(4):
                        psG = pp.tile([F1, 512], F32, tag="G", name="G")
                        for f2c in range(2):
                            sro = f2c * 520 + q * 65
                            sio = 260 + f2c * 520 + q * 65
                            nc.tensor.matmul(
                                psG[:], lhsT=S2[:, sro:sro + 65],
                                rhs=cs(f"V2B_rI{f2c}"),
                                start=(f2c == 0), stop=False)
                            nc.tensor.matmul(
                                psG[:], lhsT=S2[:, sio:sio + 65],
                                rhs=cs(f"V2B_iR{f2c}"),
                                start=False, stop=(f2c == 1))
                        eng = nc.scalar if q % 2 == 0 else nc.vector
                        _cp(eng, G2sb[:, q * 512:(q + 1) * 512], psG[:])

                    # --- itw cmul: H2 = G2 * itwB (4 q units of (ri,256)) ---
                    H2 = dp.tile([F1, 2048], BF16, tag="H32", name="H32")

                    def vg2(t, lo, hi):
                        return t[:].rearrange("p (a x) -> p a x", a=4)[:, :, lo:hi]

                    itr = cs("itwBr")[0:F1, :].rearrange(
                        "p (a x) -> p a x", a=1).to_broadcast([F1, 4, 256])
                    iti = cs("itwBi")[0:F1, :].rearrange(
                        "p (a x) -> p a x", a=1).to_broadcast([F1, 4, 256])
                    gr, gi = vg2(G2sb, 0, 256), vg2(G2sb, 256, 512)
                    hr, hi = vg2(H2, 0, 256), vg2(H2, 256, 512)
                    tvh = tmpV[:F1, 0:1024].rearrange("p (a x) -> p a x", a=4)
                    tph = tmpP[:F1, 0:1024].rearrange("p (a x) -> p a x", a=4)
                    nc.vector.tensor_tensor(hr, gr, itr, op=AT.mult)
                    nc.vector.tensor_tensor(tvh, gi, iti, op=AT.mult)
                    nc.vector.tensor_tensor(hr, hr, tvh, op=AT.subtract)
                    nc.gpsimd.tensor_tensor(hi, gr, iti, op=AT.mult)
                    nc.vector.tensor_tensor(tph, gi, itr, op=AT.mult)
                    nc.vector.tensor_tensor(hi, hi, tph, op=AT.add)

                    # --- V1B + mask/max + argmax candidates ---
                    ccm = dp.tile([128, 1024], BF16, tag="ccm", name="ccm")
                    for qp in range(2):
                        psX = pp.tile([128, 512], F32, tag="V1", name="V1")
                        hh = H2[:, qp * 1024:qp * 1024 + 1024].rearrange(
                            "p (a x) -> p a x", a=2)
                        nc.tensor.matmul(psX[:], lhsT=cs("V1Br")[0:F1, :],
                                         rhs=hh[:, :, 0:256],
                                         start=True, stop=False)
                        nc.tensor.matmul(psX[:], lhsT=cs("nV1Bi")[0:F1, :],
                                         rhs=hh[:, :, 256:512],
                                         start=False, stop=True)
                        ccv = ccm[:, qp * 512:(qp + 1) * 512].rearrange(
                            "p (a x) -> p a x", a=2)
                        mb = cs("maskB").rearrange(
                            "p (a x) -> p a x", a=1).to_broadcast([128, 2, 256])
                        nc.vector.scalar_tensor_tensor(
                            ccv, psX[:].rearrange("p (a x) -> p a x", a=2),
                            1.0, mb, op0=AT.bypass, op1=AT.add)
                        nc.vector.tensor_reduce(
                            allmax[:, r0 + qp * 2:r0 + qp * 2 + 2], ccv,
                            axis=AX.X, op=AT.max)
                    # candidates for the 4 rows in one batch
                    eqm = dp.tile([128, 1024], BF16, tag="eqm", name="eqm")
                    selm = dp.tile([128, 1024], F32, tag="selm", name="selm")
                    ccv4 = ccm[:].rearrange("p (a x) -> p a x", a=4)
                    amb = allmax[:, r0:r0 + 4].rearrange(
                        "p (a x) -> p a x", x=1).to_broadcast([128, 4, 256])
                    nc.vector.tensor_tensor(
                        eqm[:].rearrange("p (a x) -> p a x", a=4),
                        ccv4, amb, op=AT.is_equal)
                    shb = cs("shvB").rearrange(
                        "p (a x) -> p a x", a=1).to_broadcast([128, 4, 256])
                    nc.gpsimd.tensor_tensor(
                        selm[:].rearrange("p (a x) -> p a x", a=4),
                        eqm[:].rearrange("p (a x) -> p a x", a=4),
                        shb, op=AT.mult)
                    nc.vector.tensor_reduce(
                        argshv[:, r0:r0 + 4],
                        selm[:].rearrange("p (a x) -> p a x", a=4),
                        axis=AX.X, op=AT.min)

            _ps1ctx.__exit__(None, None, None)
            _ps2ctx.__exit__(None, None, None)

            # ---------------- D) argmax tail -> shifts -> loss ----------------
            with (
                tc.tile_pool(name="amax", bufs=1) as dp,
                tc.tile_pool(name="psD", bufs=1, space="PSUM") as pd,
            ):
                ptA = pd.tile([R, 128], BF16, tag="ptA", name="ptA")
                nc.tensor.transpose(ptA[:], allmax[:, 0:R], cs("ident"))
                tmaxB = dp.tile([R, 128], BF16, tag="tmaxB", name="tmaxB")
                nc.scalar.copy(tmaxB[:], ptA[:])
                rowmax = dp.tile([R, 1], BF16, tag="rowmax", name="rowmax")
                nc.vector.tensor_reduce(rowmax[:], tmaxB[:], axis=AX.X,
                                        op=AT.max)
                prm = pd.tile([1, R], BF16, tag="prm", name="prm")
                nc.tensor.transpose(prm[:], rowmax[:], cs("ident")[0:R, 0:R])
                rmT = dp.tile([1, R], BF16, tag="rmT", name="rmT")
                nc.scalar.copy(rmT[:], prm[:])
                pmb = pd.tile([128, R], F32, tag="pmb", name="pmb")
                nc.tensor.matmul(pmb[:], lhsT=cs("W1r")[0:1, :],
                                 rhs=rmT[:], start=True, stop=True)
                MbB = dp.tile([128, R], BF16, tag="MbB", name="MbB")
                nc.scalar.copy(MbB[:], pmb[:])

                eq1 = dp.tile([128, R], BF16, tag="eq1", name="eq1")
                nc.vector.tensor_tensor(eq1[:], allmax[:, 0:R], MbB[:],
                                        op=AT.is_equal)
                selA = dp.tile([128, R], F32, tag="selA", name="selA")
                nc.vector.tensor_tensor(selA[:], eq1[:], argshv[:, 0:R],
                                        op=AT.mult)
                ptS = pd.tile([R, 128], F32, tag="ptS", name="ptS")
                nc.tensor.transpose(ptS[:], selA[:], cs("identF"))
                tminS = dp.tile([R, 128], F32, tag="tminS", name="tminS")
                nc.scalar.copy(tminS[:], ptS[:])
                nc.vector.tensor_reduce(shifts[:], tminS[:], axis=AX.X,
                                        op=AT.min)
                nc.vector.tensor_scalar_add(shifts[:], shifts[:],
                                            BIGL + float(START0))

                # start = (7040 + shift) mod 14337
                m1 = dp.tile([R, 1], F32, tag="m1", name="m1")
                nc.vector.tensor_scalar(out=m1[:], in0=shifts[:], scalar1=0.0,
                                        scalar2=None, op0=AT.is_lt)
                nc.vector.scalar_tensor_tensor(
                    shifts[:], m1[:], float(CONV_LEN), shifts[:],
                    op0=AT.mult, op1=AT.add)
                nc.vector.tensor_scalar(out=m1[:], in0=shifts[:],
                                        scalar1=float(CONV_LEN), scalar2=None,
                                        op0=AT.is_ge)
                nc.vector.scalar_tensor_tensor(
                    shifts[:], m1[:], float(-CONV_LEN), shifts[:],
                    op0=AT.mult, op1=AT.add)

                idxf = dp.tile([R, CROP], F32, tag="idxf", name="idxf")
                nc.vector.tensor_tensor(idxf[:], cs("winidx")[0:R, :],
                                        shifts[:].to_broadcast([R, CROP]),
                                        op=AT.add)
                idxi = dp.tile([R, CROP], I32, tag="idxi", name="idxi")
                nc.vector.tensor_copy(idxi[:], idxf[:])
                w = dp.tile([R, CROP], BF16, tag="wg", name="wg")
                nc.gpsimd.indirect_dma_start(
                    out=w[:], out_offset=None,
                    in_=scratch.ap().rearrange("r p -> (r p)").rearrange(
                        "(a b) -> a b", b=1),
                    in_offset=bass.IndirectOffsetOnAxis(ap=idxi[:], axis=0),
                )
                tw_ = dp.tile([R, CROP], F32, tag="twin", name="twin")
                nc.sync.dma_start(tw_[:], target[:, START0:START0 + CROP])
                nc.vector.tensor_tensor(w[:], w[:], tw_[:], op=AT.subtract)
                convacc = dp.tile([R, 1], F32, tag="convacc", name="convacc")
                nc.vector.scalar_tensor_tensor(
                    tw_[:], w[:], 1.0, w[:], op0=AT.bypass, op1=AT.mult,
                    accum_out=convacc[:])

                a0 = dp.tile([128, 1], F32, tag="a0", name="a0")
                nc.vector.tensor_reduce(a0[:], astf_acc[:], axis=AX.X,
                                        op=AT.add)
                psa = pd.tile([1, 1], F32, tag="psa", name="psa")
                nc.tensor.matmul(psa[:], lhsT=a0[:], rhs=cs("ones128"),
                                 start=True, stop=True)
                psc = pd.tile([1, 1], F32, tag="psc", name="psc")
                nc.tensor.matmul(psc[:], lhsT=convacc[:],
                                 rhs=cs("ones64")[0:R, :],
                                 start=True, stop=True)
                nc.scalar.copy(outt[:, 0:1], psa[:])
                nc.scalar.copy(outt[:, 1:2], psc[:])
                nc.sync.dma_start(out[:], outt[:])

    nc.finalize()
    return nc


_CACHE = {}


def get_built():
    if "nc" not in _CACHE:
        _CACHE["nc"] = build_nc()
        _CACHE["consts"] = make_packed_consts()[0]
    return _CACHE["nc"], _CACHE["consts"]


LAST_RESULT = {}


def kernel(pred_astf, true_astf, egf, target_waveform):
    import os
    from concourse.bass_utils import run_bass_kernel_spmd
    nc, consts = get_built()
    pred_astf = np.ascontiguousarray(np.asarray(pred_astf, np.float32))
    true_astf = np.ascontiguousarray(np.asarray(true_astf, np.float32))
    egf = np.ascontiguousarray(np.asarray(egf, np.float32))
    target_waveform = np.ascontiguousarray(
        np.asarray(target_waveform, np.float32))
    B = pred_astf.shape[0]
    per = B // NCORES
    in_maps = []
    for i in range(NCORES):
        sl = slice(i * per, (i + 1) * per)
        m = {"pred": pred_astf[sl], "true": true_astf[sl],
             "egf": egf[sl], "target": target_waveform[sl]}
        m.update(consts)
        in_maps.append(m)
    trace = os.environ.get("CONVALIGN_TRACE") == "1"
    res = run_bass_kernel_spmd(nc, in_maps, core_ids=list(range(NCORES)),
                               trace=trace)
    LAST_RESULT["res"] = res
    sums = np.stack([res.results[i]["out"][0] for i in range(NCORES)])
    loss_astf = np.float32(sums[:, 0].sum() / (B * L1))
    loss_conv = np.float32(sums[:, 1].sum() / (B * CROP))
    total = np.float32(loss_astf + loss_conv)
    return total, loss_astf, loss_conv
